# revision 1
# baseline (speedup 1.0000x reference)
"""Causal grouped-query paged attention (prefill) on 8 Trainium2 NeuronCores.

Problem (hardcoded): T=4096 tokens (B=2 seqs x SEQ=2048), 32 q heads,
8 kv heads (GQA group g=4), head_dim=128, paged fp32 KV cache
(512 blocks x 16 tokens).

Sharding: tensor-parallel over KV heads -- core h gets kv head h, its 4
query heads, and both sequences => 8 causal attention slices of
[2048 q x 2048 k x 128 d] per core.

Device kernel design (per core):
  - S^T orientation: scores^T[k, q] = (K_chunk as lhsT).T-free matmul with
    Q^T as the moving operand, so k lands on PSUM partitions and the P@V
    matmul needs no transposes (P^T chunks are the moving operand against
    V chunks [k, d] as stationary).
  - exp on ScalarE (scale folded in), causal masking only on diagonal
    128x128 triangles via a precomputed 0/1 mask (VectorE multiply).
  - softmax denominator: ones[128,1] stationary matmul over P^T, PSUM-
    accumulated row [1, 512]; transposed to a column via PE transpose.
  - O^T accumulated in PSUM [d, 512], copied to SBUF, PE-transposed to
    [q, d] tiles, normalized by 1/denom (per-partition scalar on VectorE),
    DMA'd out token-major.
  - all matmuls in fp32r (full PE rate at moving dim >= 256).

kernel(**inputs) does the paged cache scatter/gather + head sharding +
layout transposes host-side in numpy (pure data movement), runs the same
NEFF SPMD on cores 0-7, and reassembles the full [4096, 4096] output.
"""

import math

import numpy as np

import concourse.bass as bass
import concourse.tile as tile
from concourse import bacc, mybir
from concourse.bass_utils import run_bass_kernel_spmd
from concourse.masks import make_identity

# problem constants
B = 2
SEQ = 2048
T = B * SEQ
N_QO_HEADS = 32
N_KV_HEADS = 8
G = N_QO_HEADS // N_KV_HEADS  # 4
D = 128
PAGE = 16
NUM_BLOCKS = 512
N_CORES = 8

QTILE = 512  # q chunk (matmul moving dim)
KCH = 128    # k chunk (contraction tile)
F32 = mybir.dt.float32
F32R = mybir.dt.float32r
BF16 = mybir.dt.bfloat16
FP16 = mybir.dt.float16
SM_SCALE = 1.0 / math.sqrt(D)


def emit(nc, n_slices, n_seqs, seq, slice_to_seq, debug=False):
    """Emit the attention program. Inputs (DRAM):
      qt   [n_slices, 128, seq]  Q^T per slice
      kt   [n_seqs,   128, seq]  K^T per sequence
      v    [n_seqs,   seq, 128]  V per sequence
      mask [128, 256]            cols 0:128 zeros, 128:256 upper-tri ones
    Output: o [n_slices, seq, 128]
    """
    nq = seq // QTILE
    ndiag = QTILE // KCH  # 4

    qt = nc.dram_tensor("qt", [n_slices, D, seq], FP16, kind="ExternalInput").ap()
    kt = nc.dram_tensor("kt", [n_seqs, D, seq], FP16, kind="ExternalInput").ap()
    v = nc.dram_tensor("v", [n_seqs, seq, D], FP16, kind="ExternalInput").ap()
    mask = nc.dram_tensor("mask", [D, 256], FP16, kind="ExternalInput").ap()
    ones = nc.dram_tensor("ones", [D, D], FP16, kind="ExternalInput").ap()
    o = nc.dram_tensor("o", [n_slices, seq, D], F32, kind="ExternalOutput").ap()
    if debug:
        dbg_dsrow = nc.dram_tensor("dbg_dsrow", [1, QTILE], F32, kind="ExternalOutput").ap()
        dbg_row4 = nc.dram_tensor("dbg_row4", [4, D], F32, kind="ExternalOutput").ap()
        dbg_rd4 = nc.dram_tensor("dbg_rd4", [D, 4], F32, kind="ExternalOutput").ap()
        dbg_pt = nc.dram_tensor("dbg_pt", [D, QTILE], FP16, kind="ExternalOutput").ap()
        dbg_ot = nc.dram_tensor("dbg_ot", [D, D], FP16, kind="ExternalOutput").ap()

    with tile.TileContext(nc) as tc:
        with (
            tc.tile_pool(name="const", bufs=1) as const_pool,
            tc.tile_pool(name="kv", bufs=1) as kv_pool,
            tc.tile_pool(name="q", bufs=1) as q_pool,
            tc.tile_pool(name="pt", bufs=4) as pt_pool,
            tc.tile_pool(name="otsb", bufs=2) as otsb_pool,
            tc.tile_pool(name="osb", bufs=4) as osb_pool,
            tc.tile_pool(name="rd", bufs=4) as rd_pool,
            tc.tile_pool(name="st", bufs=2, space="PSUM") as st_pool,
            tc.tile_pool(name="ot", bufs=2, space="PSUM") as ot_pool,
            tc.tile_pool(name="ds", bufs=2, space="PSUM") as ds_pool,
            tc.tile_pool(name="tr", bufs=2, space="PSUM") as tr_pool,
        ):
            identity = const_pool.tile([D, D], F32)
            make_identity(nc, identity[:])
            identity_h = const_pool.tile([D, D], FP16)
            make_identity(nc, identity_h[:])
            mask_sb = const_pool.tile([D, 256], FP16)
            nc.sync.dma_start(mask_sb[:], mask[:])
            ones_sb = const_pool.tile([D, D], FP16)
            nc.sync.dma_start(ones_sb[:], ones[:])
            kt_sb = []
            v_sb = []
            for b in range(n_seqs):
                kt_sb.append(kv_pool.tile([D, seq], FP16, tag=f"kt{b}", name=f"ktsb{b}"))
                v_sb.append(kv_pool.tile([D, seq], FP16, tag=f"v{b}", name=f"vsb{b}"))
            qt_sb = [
                q_pool.tile([D, seq], FP16, tag=f"qt{s}", name=f"qtsb{s}")
                for s in range(n_slices)
            ]
            # load order: everything slice 0 needs first, then the rest
            b0 = slice_to_seq[0]
            loaded = set()

            def load_seq(b):
                if b in loaded:
                    return
                loaded.add(b)
                nc.sync.dma_start(kt_sb[b][:], kt[b])
                # v chunks packed along free dim: chunk c at cols [c*128, +128)
                nc.sync.dma_start(
                    v_sb[b][:].rearrange("p (c d) -> p c d", d=D),
                    v[b].rearrange("(c p) d -> p c d", p=D),
                )

            load_seq(b0)
            nc.sync.dma_start(qt_sb[0][:], qt[0])
            for b in range(n_seqs):
                load_seq(b)
            for s in range(1, n_slices):
                nc.sync.dma_start(qt_sb[s][:], qt[s])

            for s in range(n_slices):
                b = slice_to_seq[s]
                for qc in range(nq):
                    # (kc, col offset, width, mask kind)
                    chunks = [(kc, 0, QTILE, None) for kc in range(ndiag * qc)]
                    base = ndiag * qc
                    chunks += [
                        (base + 0, 0, 512, "tri"),
                        (base + 1, 128, 384, "tri"),
                        (base + 2, 256, 256, "tri"),
                        (base + 3, 256, 256, "combo"),
                    ]
                    ot_ps = ot_pool.tile([D, QTILE], F32)
                    ds_ps = ds_pool.tile([D, QTILE], F32)
                    nchunks = len(chunks)
                    for i, (kc, off, w, mk) in enumerate(chunks):
                        st_ps = st_pool.tile([D, QTILE], F32)
                        nc.tensor.matmul(
                            st_ps[:, :w],
                            lhsT=kt_sb[b][:, kc * KCH : (kc + 1) * KCH],
                            rhs=qt_sb[s][:, qc * QTILE + off : qc * QTILE + off + w],
                            start=True,
                            stop=True,
                        )
                        ptile = pt_pool.tile([D, QTILE], FP16)
                        nc.scalar.activation(
                            ptile[:, :w],
                            st_ps[:, :w],
                            mybir.ActivationFunctionType.Exp,
                            scale=SM_SCALE,
                        )
                        if debug and s == 0 and qc == 0 and i == 0:
                            nc.sync.dma_start(dbg_pt[:], ptile[:])
                        if mk == "tri":
                            nc.vector.tensor_mul(
                                ptile[:, 0:128], ptile[:, 0:128], mask_sb[:, 128:256]
                            )
                        elif mk == "combo":
                            nc.vector.tensor_mul(
                                ptile[:, 0:256], ptile[:, 0:256], mask_sb[:, 0:256]
                            )
                        first = i == 0
                        last = i == nchunks - 1
                        nc.tensor.matmul(
                            ot_ps[:, off : off + w],
                            lhsT=v_sb[b][:, kc * KCH : (kc + 1) * KCH],
                            rhs=ptile[:, :w],
                            start=first,
                            stop=last,
                        )
                        nc.tensor.matmul(
                            ds_ps[:, off : off + w],
                            lhsT=ones_sb[:],
                            rhs=ptile[:, :w],
                            start=first,
                            stop=last,
                        )

                    # epilogue: denominators row -> column (DMA reshape +
                    # one PE transpose), then transpose O^T -> O, normalize.
                    dsrow = rd_pool.tile([1, QTILE], F32, tag="dsrow")
                    nc.vector.tensor_copy(dsrow[0:1, :], ds_ps[0:1, :])
                    row4 = rd_pool.tile([4, D], F32, tag="row4")
                    for j in range(4):
                        nc.sync.dma_start(
                            row4[j : j + 1, :], dsrow[0:1, j * D : (j + 1) * D]
                        )
                    if debug and s == 0 and qc == 0:
                        nc.sync.dma_start(dbg_dsrow[:], dsrow[0:1, :])
                        nc.sync.dma_start(dbg_row4[:], row4[0:4, :])
                    dcol4 = tr_pool.tile([D, D], F32, tag="tr")
                    nc.tensor.transpose(dcol4[:, 0:4], row4[0:4, 0:D], identity[0:4, 0:4])
                    rd4 = rd_pool.tile([D, 4], F32, tag="rd4")
                    nc.vector.reciprocal(rd4[:], dcol4[:, 0:4])
                    if debug and s == 0 and qc == 0:
                        nc.sync.dma_start(dbg_rd4[:], rd4[:])
                    for qi in range(QTILE // D):
                        ot_sb = otsb_pool.tile([D, D], FP16, tag="ot_sb")
                        nc.vector.tensor_copy(
                            ot_sb[:], ot_ps[:, qi * D : (qi + 1) * D]
                        )
                        o_ps16 = tr_pool.tile([D, D], FP16, tag="tr")
                        nc.tensor.transpose(o_ps16[:], ot_sb[:], identity_h[:])
                        o_sb = osb_pool.tile([D, D], F32, tag="o_sb")
                        nc.vector.tensor_scalar_mul(
                            o_sb[:], o_ps16[:], rd4[:, qi : qi + 1]
                        )
                        if debug and s == 0 and qc == 0 and qi == 0:
                            nc.sync.dma_start(dbg_ot[:], ot_sb[:])
                        q0 = qc * QTILE + qi * D
                        nc.sync.dma_start(o[s, q0 : q0 + D, :], o_sb[:])
    return nc


_CACHE = {}


def _build_full():
    key = "full"
    if key not in _CACHE:
        nc = bacc.Bacc(
            "TRN2",
            target_bir_lowering=False,
            debug=False,
            enable_asserts=False,
            num_devices=N_CORES,
        )
        emit(nc, n_slices=B * G, n_seqs=B, seq=SEQ,
             slice_to_seq=[b for b in range(B) for _ in range(G)])
        nc.compile()
        _CACHE[key] = nc
    return _CACHE[key]


def make_mask():
    m = np.zeros((D, 256), dtype=np.float16)
    m[:, 128:256] = np.triu(np.ones((D, D), dtype=np.float16))
    return m


def shard_inputs(query, key, value, key_cache, value_cache, block_tables,
                 new_cache_slots):
    """Host-side scatter/gather + head sharding. Returns per-core input maps."""
    kc = key_cache.reshape(NUM_BLOCKS * PAGE, N_KV_HEADS, D).copy()
    vc = value_cache.reshape(NUM_BLOCKS * PAGE, N_KV_HEADS, D).copy()
    kc[new_cache_slots] = key.reshape(T, N_KV_HEADS, D)
    vc[new_cache_slots] = value.reshape(T, N_KV_HEADS, D)
    nb = block_tables.shape[1]
    idx = (
        block_tables[:, :, None].astype(np.int64) * PAGE
        + np.arange(PAGE, dtype=np.int64)[None, None, :]
    ).reshape(B, SEQ)
    k_all = kc[idx]  # [B, SEQ, Hkv, D]
    v_all = vc[idx]
    q_all = query.reshape(B, SEQ, N_KV_HEADS, G, D)
    mask = make_mask()

    bf = np.float16
    in_maps = []
    for h in range(N_CORES):
        qt = np.ascontiguousarray(
            q_all[:, :, h, :, :].transpose(0, 2, 3, 1).reshape(B * G, D, SEQ)
        ).astype(bf)
        kt = np.ascontiguousarray(k_all[:, :, h, :].transpose(0, 2, 1)).astype(bf)
        vv = np.ascontiguousarray(v_all[:, :, h, :]).astype(bf)
        in_maps.append({"qt": qt, "kt": kt, "v": vv, "mask": mask,
                        "ones": np.ones((D, D), dtype=bf)})
    return in_maps


def assemble_output(results):
    out = np.empty((B, SEQ, N_KV_HEADS, G, D), dtype=np.float32)
    for h in range(N_CORES):
        oc = results[h]["o"].reshape(B, G, SEQ, D)
        out[:, :, h, :, :] = oc.transpose(0, 2, 1, 3)
    return out.reshape(T, N_QO_HEADS * D)


def kernel(query, key, value, key_cache, value_cache, block_tables,
           new_cache_slots, _trace=False):
    query = np.asarray(query, dtype=np.float32)
    key = np.asarray(key, dtype=np.float32)
    value = np.asarray(value, dtype=np.float32)
    key_cache = np.asarray(key_cache, dtype=np.float32)
    value_cache = np.asarray(value_cache, dtype=np.float32)
    block_tables = np.asarray(block_tables)
    new_cache_slots = np.asarray(new_cache_slots)

    nc = _build_full()
    in_maps = shard_inputs(query, key, value, key_cache, value_cache,
                           block_tables, new_cache_slots)
    res = run_bass_kernel_spmd(
        nc, in_maps, core_ids=list(range(N_CORES)), trace=_trace
    )
    out = assemble_output(res.results)
    if _trace:
        kernel.last_result = res
    return out



# revision 5
# speedup vs baseline: 1.3296x; 1.3296x over previous
"""Causal grouped-query paged attention (prefill) on 8 Trainium2 NeuronCores.

Problem (hardcoded): T=4096 tokens (B=2 seqs x SEQ=2048), 32 q heads,
8 kv heads (GQA group g=4), head_dim=128, paged fp32 KV cache
(512 blocks x 16 tokens).

Sharding: tensor-parallel over KV heads -- core h gets kv head h, its 4
query heads, and both sequences => 8 causal attention slices of
[2048 q x 2048 k x 128 d] per core.

v2 design notes (vs v1 baseline at ~300us):
  - S^T orientation: scores^T[k, q] via K-chunk-stationary matmuls, so the
    P@V matmul needs no transposes.
  - The softmax denominator matmul (ones^T @ P per chunk, 1/3 of all PE
    streaming in v1) is gone: DVE accumulates P chunks elementwise into a
    per-row fp16 acc[128, 512]; the 128-partition reduction and the
    1/denominator normalize both happen host-side on the DMA'd acc.
  - O^T is DMA'd out untransposed and unnormalized (fp32); host transposes.
    This removes all PE transposes and the v1 epilogue dependency chain.
  - exp instructions merged: QK scores for up to 3 k-chunks land in one
    [128, 1536] PSUM tile (3 banks, double-buffered) and get one ScalarE
    activation; ScalarE per-instruction overhead (~190ns) was ~40% of its
    busy time in v1.
  - diagonal (masked) chunks pack gapless into one [128, 1280] extent:
    j0@0 w512 | j1@512 w384 | j3@896 w128 | j2@1024 w256, so one exp and
    three DVE tri-mask multiplies cover the whole diagonal of a q-row.
  - PSUM->SBUF O^T copies stay on DVE (GpSimd has no PSUM port); the
    diagonal tri-mask multiplies go to the otherwise-idle GpSimd.
"""

import math

import numpy as np

import concourse.bass as bass
import concourse.tile as tile
from concourse import bacc, mybir
from concourse.bass_utils import run_bass_kernel_spmd

# problem constants
B = 2
SEQ = 2048
T = B * SEQ
N_QO_HEADS = 32
N_KV_HEADS = 8
G = N_QO_HEADS // N_KV_HEADS  # 4
D = 128
PAGE = 16
NUM_BLOCKS = 512
N_CORES = 8

QTILE = 512   # q chunk (matmul moving dim)
KCH = 128     # k chunk (contraction tile)
GRP = 3       # full k-chunks per PSUM score tile / exp instruction
STW = GRP * QTILE  # 1536 cols = 3 PSUM banks
F32 = mybir.dt.float32
FP16 = mybir.dt.float16
SM_SCALE = 1.0 / math.sqrt(D)

# diagonal packing: (j, st_off, width, qoff) in emission order; gapless
# within [0, 1280) and each chunk's columns stay inside one PSUM bank.
DIAG = [
    (0, 0, 512, 0),
    (1, 512, 384, 128),
    (3, 896, 128, 384),
    (2, 1024, 256, 256),
]
DIAG_EXT = 1280
# tri-mask multiply positions in the packed layout: (st_off, width);
# [896,1152) covers j3's 128 cols plus j2's first 128 via mask2 = tri|tri.
DIAG_MASKS = [(0, 128), (512, 128), (896, 256)]


def emit(nc, n_slices, n_seqs, seq, slice_to_seq):
    """Emit the attention program. Inputs (DRAM):
      qt    [n_slices, 128, seq]  Q^T per slice
      kt    [n_seqs,   128, seq]  K^T per sequence
      v     [n_seqs,   seq, 128]  V per sequence
      mask  [128, 128]            upper-tri 0/1
      mask2 [128, 256]            [tri | tri]
    Outputs:
      ot [n_slices, 128, seq]        O^T, unnormalized, fp32
      ds [n_slices, nq, 128, QTILE]  per-row P partial sums (fp16); host
                                     reduces partitions -> denominators
    """
    nq = seq // QTILE

    qt = nc.dram_tensor("qt", [n_slices, D, seq], FP16, kind="ExternalInput").ap()
    kt = nc.dram_tensor("kt", [n_seqs, D, seq], FP16, kind="ExternalInput").ap()
    v = nc.dram_tensor("v", [n_seqs, seq, D], FP16, kind="ExternalInput").ap()
    mask = nc.dram_tensor("mask", [D, D], FP16, kind="ExternalInput").ap()
    mask2 = nc.dram_tensor("mask2", [D, 256], FP16, kind="ExternalInput").ap()
    ot = nc.dram_tensor("ot", [n_slices, D, seq], F32, kind="ExternalOutput").ap()
    ds = nc.dram_tensor(
        "ds", [n_slices, nq, D, QTILE], FP16, kind="ExternalOutput"
    ).ap()

    with tile.TileContext(nc) as tc:
        with (
            tc.tile_pool(name="const", bufs=1) as const_pool,
            tc.tile_pool(name="kv", bufs=1) as kv_pool,
            tc.tile_pool(name="q", bufs=1) as q_pool,
            tc.tile_pool(name="pt", bufs=3) as pt_pool,
            tc.tile_pool(name="acc", bufs=2) as acc_pool,
            tc.tile_pool(name="osb", bufs=2) as osb_pool,
            tc.tile_pool(name="st", bufs=2, space="PSUM") as st_pool,
            tc.tile_pool(name="ot", bufs=2, space="PSUM") as ot_pool,
        ):
            mask_sb = const_pool.tile([D, D], FP16)
            nc.sync.dma_start(mask_sb[:], mask[:])
            mask2_sb = const_pool.tile([D, 256], FP16)
            nc.sync.dma_start(mask2_sb[:], mask2[:])
            kt_sb = []
            v_sb = []
            for b in range(n_seqs):
                kt_sb.append(kv_pool.tile([D, seq], FP16, tag=f"kt{b}", name=f"ktsb{b}"))
                v_sb.append(kv_pool.tile([D, seq], FP16, tag=f"v{b}", name=f"vsb{b}"))
            qt_sb = [
                q_pool.tile([D, seq], FP16, tag=f"qt{s}", name=f"qtsb{s}")
                for s in range(n_slices)
            ]
            b0 = slice_to_seq[0]
            loaded = set()

            def load_seq(b):
                if b in loaded:
                    return
                loaded.add(b)
                nc.sync.dma_start(kt_sb[b][:], kt[b])
                # v chunks packed along free dim: chunk c at cols [c*128, +128)
                nc.sync.dma_start(
                    v_sb[b][:].rearrange("p (c d) -> p c d", d=D),
                    v[b].rearrange("(c p) d -> p c d", p=D),
                )

            load_seq(b0)
            nc.sync.dma_start(qt_sb[0][:], qt[0])
            for b in range(n_seqs):
                load_seq(b)
            for s in range(1, n_slices):
                nc.sync.dma_start(qt_sb[s][:], qt[s])

            for s in range(n_slices):
                b = slice_to_seq[s]
                for qc in range(nq):
                    nfull = (QTILE // KCH) * qc
                    ot_ps = ot_pool.tile([D, QTILE], F32)
                    acc = acc_pool.tile([D, QTILE], FP16)
                    first = True  # first PV matmul / first ds op of the row

                    def do_group(kcs, offs, widths, qoffs, masked):
                        nonlocal first
                        ext = offs[-1] + widths[-1]
                        st = st_pool.tile([D, STW], F32, tag="st", name="st")
                        for kc, off, w, qoff in zip(kcs, offs, widths, qoffs):
                            nc.tensor.matmul(
                                st[:, off : off + w],
                                lhsT=kt_sb[b][:, kc * KCH : (kc + 1) * KCH],
                                rhs=qt_sb[s][
                                    :, qc * QTILE + qoff : qc * QTILE + qoff + w
                                ],
                                start=True,
                                stop=True,
                            )
                        pt = pt_pool.tile([D, STW], FP16, tag="pt", name="pt")
                        nc.scalar.activation(
                            pt[:, :ext],
                            st[:, :ext],
                            mybir.ActivationFunctionType.Exp,
                            scale=SM_SCALE,
                        )
                        if masked:
                            # tri masks on GpSimd (SBUF-only engine, else idle)
                            for moff, mw in DIAG_MASKS:
                                msk = mask_sb if mw == 128 else mask2_sb
                                nc.gpsimd.tensor_mul(
                                    pt[:, moff : moff + mw],
                                    pt[:, moff : moff + mw],
                                    msk[:, :mw],
                                )
                        n = len(kcs)
                        for i, (kc, off, w, qoff) in enumerate(
                            zip(kcs, offs, widths, qoffs)
                        ):
                            last_pv = masked and i == n - 1
                            nc.tensor.matmul(
                                ot_ps[:, qoff : qoff + w],
                                lhsT=v_sb[b][:, kc * KCH : (kc + 1) * KCH],
                                rhs=pt[:, off : off + w],
                                start=first,
                                stop=last_pv,
                            )
                            if first:
                                nc.vector.tensor_copy(
                                    acc[:, qoff : qoff + w], pt[:, off : off + w]
                                )
                            else:
                                nc.vector.tensor_add(
                                    acc[:, qoff : qoff + w],
                                    acc[:, qoff : qoff + w],
                                    pt[:, off : off + w],
                                )
                            first = False

                    # full chunks in groups of GRP
                    for g0 in range(0, nfull, GRP):
                        kcs = list(range(g0, min(g0 + GRP, nfull)))
                        do_group(
                            kcs,
                            [i * QTILE for i in range(len(kcs))],
                            [QTILE] * len(kcs),
                            [0] * len(kcs),
                            masked=False,
                        )
                    # diagonal chunks, packed gapless
                    do_group(
                        [4 * qc + j for j, _, _, _ in DIAG],
                        [off for _, off, _, _ in DIAG],
                        [w for _, _, w, _ in DIAG],
                        [qoff for _, _, _, qoff in DIAG],
                        masked=True,
                    )

                    # epilogue: O^T out via DVE copy + DMA; acc out raw
                    # (GpSimd has no PSUM port)
                    ot_sb = osb_pool.tile([D, QTILE], F32, tag="ot_sb")
                    nc.vector.tensor_copy(ot_sb[:], ot_ps[:])
                    nc.sync.dma_start(
                        ot[s][:, qc * QTILE : (qc + 1) * QTILE], ot_sb[:]
                    )
                    nc.sync.dma_start(ds[s, qc], acc[:])
    return nc


_CACHE = {}


def _build_full():
    key = "full"
    if key not in _CACHE:
        nc = bacc.Bacc(
            "TRN2",
            target_bir_lowering=False,
            debug=False,
            enable_asserts=False,
            num_devices=N_CORES,
        )
        emit(nc, n_slices=B * G, n_seqs=B, seq=SEQ,
             slice_to_seq=[b for b in range(B) for _ in range(G)])
        nc.compile()
        _CACHE[key] = nc
    return _CACHE[key]


def make_masks():
    tri = np.triu(np.ones((D, D), dtype=np.float16))
    return tri, np.concatenate([tri, tri], axis=1)


def shard_inputs(query, key, value, key_cache, value_cache, block_tables,
                 new_cache_slots):
    """Host-side scatter/gather + head sharding. Returns per-core input maps."""
    kc = key_cache.reshape(NUM_BLOCKS * PAGE, N_KV_HEADS, D).copy()
    vc = value_cache.reshape(NUM_BLOCKS * PAGE, N_KV_HEADS, D).copy()
    kc[new_cache_slots] = key.reshape(T, N_KV_HEADS, D)
    vc[new_cache_slots] = value.reshape(T, N_KV_HEADS, D)
    idx = (
        block_tables[:, :, None].astype(np.int64) * PAGE
        + np.arange(PAGE, dtype=np.int64)[None, None, :]
    ).reshape(B, SEQ)
    k_all = kc[idx]  # [B, SEQ, Hkv, D]
    v_all = vc[idx]
    q_all = query.reshape(B, SEQ, N_KV_HEADS, G, D)
    mask, mask2 = make_masks()

    bf = np.float16
    in_maps = []
    for h in range(N_CORES):
        qt = np.ascontiguousarray(
            q_all[:, :, h, :, :].transpose(0, 2, 3, 1).reshape(B * G, D, SEQ)
        ).astype(bf)
        kt = np.ascontiguousarray(k_all[:, :, h, :].transpose(0, 2, 1)).astype(bf)
        vv = np.ascontiguousarray(v_all[:, :, h, :]).astype(bf)
        in_maps.append({"qt": qt, "kt": kt, "v": vv, "mask": mask, "mask2": mask2})
    return in_maps


def assemble_output(results):
    out = np.empty((B, SEQ, N_KV_HEADS, G, D), dtype=np.float32)
    for h in range(N_CORES):
        o_t = results[h]["ot"]  # [B*G, D, SEQ] f32, unnormalized
        dsv = results[h]["ds"]  # [B*G, nq, D, QTILE] f16
        den = dsv.astype(np.float32).sum(axis=2).reshape(B * G, SEQ)
        o = o_t / den[:, None, :]             # [B*G, D, SEQ]
        oc = o.reshape(B, G, D, SEQ).transpose(0, 3, 1, 2)  # [B, SEQ, G, D]
        out[:, :, h, :, :] = oc
    return out.reshape(T, N_QO_HEADS * D)


def kernel(query, key, value, key_cache, value_cache, block_tables,
           new_cache_slots, _trace=False):
    query = np.asarray(query, dtype=np.float32)
    key = np.asarray(key, dtype=np.float32)
    value = np.asarray(value, dtype=np.float32)
    key_cache = np.asarray(key_cache, dtype=np.float32)
    value_cache = np.asarray(value_cache, dtype=np.float32)
    block_tables = np.asarray(block_tables)
    new_cache_slots = np.asarray(new_cache_slots)

    nc = _build_full()
    in_maps = shard_inputs(query, key, value, key_cache, value_cache,
                           block_tables, new_cache_slots)
    res = run_bass_kernel_spmd(
        nc, in_maps, core_ids=list(range(N_CORES)), trace=_trace
    )
    out = assemble_output(res.results)
    if _trace:
        kernel.last_result = res
    return out


# revision 7
# speedup vs baseline: 1.4106x; 1.0610x over previous
"""Causal grouped-query paged attention (prefill) on 8 Trainium2 NeuronCores.

Problem (hardcoded): T=4096 tokens (B=2 seqs x SEQ=2048), 32 q heads,
8 kv heads (GQA group g=4), head_dim=128, paged fp32 KV cache
(512 blocks x 16 tokens).

Sharding: tensor-parallel over KV heads -- core h gets kv head h, its 4
query heads, and both sequences => 8 causal attention slices of
[2048 q x 2048 k x 128 d] per core.

v2 design notes (vs v1 baseline at ~300us):
  - S^T orientation: scores^T[k, q] via K-chunk-stationary matmuls, so the
    P@V matmul needs no transposes.
  - The softmax denominator matmul (ones^T @ P per chunk, 1/3 of all PE
    streaming in v1) is gone: DVE accumulates P chunks elementwise into a
    per-row fp16 acc[128, 512]; the 128-partition reduction and the
    1/denominator normalize both happen host-side on the DMA'd acc.
  - O^T is DMA'd out untransposed and unnormalized (fp32); host transposes.
    This removes all PE transposes and the v1 epilogue dependency chain.
  - exp instructions merged: QK scores for up to 3 k-chunks land in one
    [128, 1536] PSUM tile (3 banks, double-buffered) and get one ScalarE
    activation; ScalarE per-instruction overhead (~190ns) was ~40% of its
    busy time in v1.
  - diagonal (masked) chunks pack gapless into one [128, 1280] extent:
    j0@0 w512 | j1@512 w384 | j3@896 w128 | j2@1024 w256, so one exp and
    three DVE tri-mask multiplies cover the whole diagonal of a q-row.
  - PSUM->SBUF O^T copies stay on DVE (GpSimd has no PSUM port); the
    diagonal tri-mask multiplies go to the otherwise-idle GpSimd.
"""

import math

import numpy as np

import concourse.bass as bass
import concourse.tile as tile
from concourse import bacc, mybir
from concourse.bass_utils import run_bass_kernel_spmd

# problem constants
B = 2
SEQ = 2048
T = B * SEQ
N_QO_HEADS = 32
N_KV_HEADS = 8
G = N_QO_HEADS // N_KV_HEADS  # 4
D = 128
PAGE = 16
NUM_BLOCKS = 512
N_CORES = 8

QTILE = 512   # q chunk (matmul moving dim)
KCH = 128     # k chunk (contraction tile)
GRP = 3       # full k-chunks per PSUM score tile / exp instruction
STW = GRP * QTILE  # 1536 cols = 3 PSUM banks
F32 = mybir.dt.float32
FP16 = mybir.dt.float16
SM_SCALE = 1.0 / math.sqrt(D)

# diagonal packing: (j, st_off, width, qoff) in emission order; gapless
# within [0, 1280) and each chunk's columns stay inside one PSUM bank.
DIAG = [
    (0, 0, 512, 0),
    (1, 512, 384, 128),
    (3, 896, 128, 384),
    (2, 1024, 256, 256),
]
DIAG_EXT = 1280
# tri-mask multiply positions in the packed layout: (st_off, width);
# [896,1152) covers j3's 128 cols plus j2's first 128 via mask2 = tri|tri.
DIAG_MASKS = [(0, 128), (512, 128), (896, 256)]


def emit(nc, n_slices, n_seqs, seq, slice_to_seq):
    """Emit the attention program. Inputs (DRAM):
      qt    [n_slices, 128, seq]  Q^T per slice
      kt    [n_seqs,   128, seq]  K^T per sequence
      v     [n_seqs,   seq, 128]  V per sequence
      mask  [128, 128]            upper-tri 0/1
      mask2 [128, 256]            [tri | tri]
    Outputs:
      ot [n_slices, 128, seq]        O^T, unnormalized, fp32
      ds [n_slices, nq, 128, QTILE]  per-row P partial sums (fp16); host
                                     reduces partitions -> denominators
    """
    nq = seq // QTILE

    qt = nc.dram_tensor("qt", [n_slices, D, seq], FP16, kind="ExternalInput").ap()
    kt = nc.dram_tensor("kt", [n_seqs, D, seq], FP16, kind="ExternalInput").ap()
    v = nc.dram_tensor("v", [n_seqs, seq, D], FP16, kind="ExternalInput").ap()
    mask = nc.dram_tensor("mask", [D, D], FP16, kind="ExternalInput").ap()
    mask2 = nc.dram_tensor("mask2", [D, 256], FP16, kind="ExternalInput").ap()
    ot = nc.dram_tensor("ot", [n_slices, D, seq], F32, kind="ExternalOutput").ap()
    ds = nc.dram_tensor(
        "ds", [n_slices, nq, D, QTILE], FP16, kind="ExternalOutput"
    ).ap()

    with tile.TileContext(nc) as tc:
        with (
            tc.tile_pool(name="const", bufs=1) as const_pool,
            tc.tile_pool(name="kv", bufs=1) as kv_pool,
            tc.tile_pool(name="q", bufs=1) as q_pool,
            tc.tile_pool(name="pt", bufs=3) as pt_pool,
            tc.tile_pool(name="acc", bufs=2) as acc_pool,
            tc.tile_pool(name="osb", bufs=2) as osb_pool,
            tc.tile_pool(name="st", bufs=2, space="PSUM") as st_pool,
            tc.tile_pool(name="ot", bufs=2, space="PSUM") as ot_pool,
        ):
            mask_sb = const_pool.tile([D, D], FP16)
            nc.sync.dma_start(mask_sb[:], mask[:])
            mask2_sb = const_pool.tile([D, 256], FP16)
            nc.sync.dma_start(mask2_sb[:], mask2[:])
            kt_sb = []
            v_sb = []
            for b in range(n_seqs):
                kt_sb.append(kv_pool.tile([D, seq], FP16, tag=f"kt{b}", name=f"ktsb{b}"))
                v_sb.append(kv_pool.tile([D, seq], FP16, tag=f"v{b}", name=f"vsb{b}"))
            qt_sb = [
                q_pool.tile([D, seq], FP16, tag=f"qt{s}", name=f"qtsb{s}")
                for s in range(n_slices)
            ]
            b0 = slice_to_seq[0]
            loaded = set()

            def load_seq(b, split=False):
                if b in loaded:
                    return
                loaded.add(b)
                if split:
                    # halves so slice 0 row 0 can start after ~1/4 the load
                    h = seq // 2
                    nc.sync.dma_start(kt_sb[b][:, :h], kt[b][:, :h])
                    nc.sync.dma_start(kt_sb[b][:, h:], kt[b][:, h:])
                else:
                    nc.sync.dma_start(kt_sb[b][:], kt[b])
                # v chunks packed along free dim: chunk c at cols [c*128, +128)
                nc.sync.dma_start(
                    v_sb[b][:].rearrange("p (c d) -> p c d", d=D),
                    v[b].rearrange("(c p) d -> p c d", p=D),
                )

            load_seq(b0, split=True)
            nc.sync.dma_start(qt_sb[0][:], qt[0])
            for b in range(n_seqs):
                load_seq(b)
            for s in range(1, n_slices):
                nc.sync.dma_start(qt_sb[s][:], qt[s])

            # Build the global group list: per row, the diagonal (masked)
            # group FIRST, then full groups -- the GpSimd mask latency hides
            # behind the row's full-group QK/PV stream.
            # Each entry: (s, qc, kcs, offs, widths, qoffs, masked,
            #              row_first, row_last)
            sched = []
            for s in range(n_slices):
                for qc in range(nq):
                    nfull = (QTILE // KCH) * qc
                    groups = [(
                        [4 * qc + j for j, _, _, _ in DIAG],
                        [off for _, off, _, _ in DIAG],
                        [w for _, _, w, _ in DIAG],
                        [qoff for _, _, _, qoff in DIAG],
                        True,
                    )]
                    for g0 in range(0, nfull, GRP):
                        kcs = list(range(g0, min(g0 + GRP, nfull)))
                        groups.append((
                            kcs,
                            [i * QTILE for i in range(len(kcs))],
                            [QTILE] * len(kcs),
                            [0] * len(kcs),
                            False,
                        ))
                    for gi, g in enumerate(groups):
                        sched.append(
                            (s, qc, *g, gi == 0, gi == len(groups) - 1)
                        )

            # per-row live state, created at row_first, consumed at row_last
            row_state = {}

            def emit_qk_exp(ent):
                s, qc, kcs, offs, widths, qoffs, masked, rf, rl = ent
                b = slice_to_seq[s]
                if rf:
                    row_state[(s, qc)] = (
                        ot_pool.tile([D, QTILE], F32, tag="ot_ps", name="ot_ps"),
                        acc_pool.tile([D, QTILE], FP16, tag="acc", name="acc"),
                    )
                ext = offs[-1] + widths[-1]
                st = st_pool.tile([D, STW], F32, tag="st", name="st")
                for kc, off, w, qoff in zip(kcs, offs, widths, qoffs):
                    nc.tensor.matmul(
                        st[:, off : off + w],
                        lhsT=kt_sb[b][:, kc * KCH : (kc + 1) * KCH],
                        rhs=qt_sb[s][
                            :, qc * QTILE + qoff : qc * QTILE + qoff + w
                        ],
                        start=True,
                        stop=True,
                    )
                pt = pt_pool.tile([D, STW], FP16, tag="pt", name="pt")
                nc.scalar.activation(
                    pt[:, :ext],
                    st[:, :ext],
                    mybir.ActivationFunctionType.Exp,
                    scale=SM_SCALE,
                )
                if masked:
                    # tri masks on GpSimd (SBUF-only engine, else idle)
                    for moff, mw in DIAG_MASKS:
                        msk = mask_sb if mw == 128 else mask2_sb
                        nc.gpsimd.tensor_mul(
                            pt[:, moff : moff + mw],
                            pt[:, moff : moff + mw],
                            msk[:, :mw],
                        )
                return pt

            def emit_pv_ds(ent, pt):
                s, qc, kcs, offs, widths, qoffs, masked, rf, rl = ent
                b = slice_to_seq[s]
                ot_ps, acc = row_state[(s, qc)]
                n = len(kcs)
                for i, (kc, off, w, qoff) in enumerate(
                    zip(kcs, offs, widths, qoffs)
                ):
                    nc.tensor.matmul(
                        ot_ps[:, qoff : qoff + w],
                        lhsT=v_sb[b][:, kc * KCH : (kc + 1) * KCH],
                        rhs=pt[:, off : off + w],
                        start=rf and i == 0,
                        stop=rl and i == n - 1,
                    )
                    if rf and i == 0:
                        nc.vector.tensor_copy(
                            acc[:, qoff : qoff + w], pt[:, off : off + w]
                        )
                    else:
                        nc.vector.tensor_add(
                            acc[:, qoff : qoff + w],
                            acc[:, qoff : qoff + w],
                            pt[:, off : off + w],
                        )
                if rl:
                    # epilogue: O^T out via DVE copy + DMA; acc out raw
                    # (GpSimd has no PSUM port)
                    ot_sb = osb_pool.tile([D, QTILE], F32, tag="ot_sb")
                    nc.vector.tensor_copy(ot_sb[:], ot_ps[:])
                    nc.sync.dma_start(
                        ot[s][:, qc * QTILE : (qc + 1) * QTILE], ot_sb[:]
                    )
                    nc.sync.dma_start(ds[s, qc], acc[:])
                    del row_state[(s, qc)]

            # one-group software pipeline: PV/ds of group k emits after
            # QK/exp of group k+1, so the PE never waits on exp/mask latency
            pending = None
            for ent in sched:
                pt = emit_qk_exp(ent)
                if pending is not None:
                    emit_pv_ds(*pending)
                pending = (ent, pt)
            emit_pv_ds(*pending)
    return nc


_CACHE = {}


def _build_full():
    key = "full"
    if key not in _CACHE:
        nc = bacc.Bacc(
            "TRN2",
            target_bir_lowering=False,
            debug=False,
            enable_asserts=False,
            num_devices=N_CORES,
        )
        emit(nc, n_slices=B * G, n_seqs=B, seq=SEQ,
             slice_to_seq=[b for b in range(B) for _ in range(G)])
        nc.compile()
        _CACHE[key] = nc
    return _CACHE[key]


def make_masks():
    tri = np.triu(np.ones((D, D), dtype=np.float16))
    return tri, np.concatenate([tri, tri], axis=1)


def shard_inputs(query, key, value, key_cache, value_cache, block_tables,
                 new_cache_slots):
    """Host-side scatter/gather + head sharding. Returns per-core input maps."""
    kc = key_cache.reshape(NUM_BLOCKS * PAGE, N_KV_HEADS, D).copy()
    vc = value_cache.reshape(NUM_BLOCKS * PAGE, N_KV_HEADS, D).copy()
    kc[new_cache_slots] = key.reshape(T, N_KV_HEADS, D)
    vc[new_cache_slots] = value.reshape(T, N_KV_HEADS, D)
    idx = (
        block_tables[:, :, None].astype(np.int64) * PAGE
        + np.arange(PAGE, dtype=np.int64)[None, None, :]
    ).reshape(B, SEQ)
    k_all = kc[idx]  # [B, SEQ, Hkv, D]
    v_all = vc[idx]
    q_all = query.reshape(B, SEQ, N_KV_HEADS, G, D)
    mask, mask2 = make_masks()

    bf = np.float16
    in_maps = []
    for h in range(N_CORES):
        qt = np.ascontiguousarray(
            q_all[:, :, h, :, :].transpose(0, 2, 3, 1).reshape(B * G, D, SEQ)
        ).astype(bf)
        kt = np.ascontiguousarray(k_all[:, :, h, :].transpose(0, 2, 1)).astype(bf)
        vv = np.ascontiguousarray(v_all[:, :, h, :]).astype(bf)
        in_maps.append({"qt": qt, "kt": kt, "v": vv, "mask": mask, "mask2": mask2})
    return in_maps


def assemble_output(results):
    out = np.empty((B, SEQ, N_KV_HEADS, G, D), dtype=np.float32)
    for h in range(N_CORES):
        o_t = results[h]["ot"]  # [B*G, D, SEQ] f32, unnormalized
        dsv = results[h]["ds"]  # [B*G, nq, D, QTILE] f16
        den = dsv.astype(np.float32).sum(axis=2).reshape(B * G, SEQ)
        o = o_t / den[:, None, :]             # [B*G, D, SEQ]
        oc = o.reshape(B, G, D, SEQ).transpose(0, 3, 1, 2)  # [B, SEQ, G, D]
        out[:, :, h, :, :] = oc
    return out.reshape(T, N_QO_HEADS * D)


def kernel(query, key, value, key_cache, value_cache, block_tables,
           new_cache_slots, _trace=False):
    query = np.asarray(query, dtype=np.float32)
    key = np.asarray(key, dtype=np.float32)
    value = np.asarray(value, dtype=np.float32)
    key_cache = np.asarray(key_cache, dtype=np.float32)
    value_cache = np.asarray(value_cache, dtype=np.float32)
    block_tables = np.asarray(block_tables)
    new_cache_slots = np.asarray(new_cache_slots)

    nc = _build_full()
    in_maps = shard_inputs(query, key, value, key_cache, value_cache,
                           block_tables, new_cache_slots)
    res = run_bass_kernel_spmd(
        nc, in_maps, core_ids=list(range(N_CORES)), trace=_trace
    )
    out = assemble_output(res.results)
    if _trace:
        kernel.last_result = res
    return out
